# revision 32
# baseline (speedup 1.0000x reference)
"""Trainium2 Bass kernel for nn_Attention_86217173500445.

Cross-attention block: shared QKV projections over two inputs (base/target),
4 attention streams (bb, tt, bt, tb), shared output projection.

Strategy: data-parallel over batch (B=32 -> 4 per core on 8 cores), weights
replicated, zero collectives.  Per-core compute is a fused bf16 pipeline
(fp32 PSUM accumulation everywhere, tolerance is 2e-2):
  - x transposed on-chip (PE transpose, fp32) to XT [C, S] bf16.
  - Q/K projections emit transposed QT/KT [C, 2src, S] bf16; V natural
    layout with a ones column (col 64) for even heads only, so their AV
    matmuls produce softmax row-sums for free; odd heads' row-sums come
    from ones_sel (all-ones lhsT column) matmuls accumulating straight
    into the rsum PSUM tile.
  - ScoresT [k, q] per (head, j), j-major; ACT exp -> bf16 feeds AV.
  - Odd heads' AV writes PSUM partitions 64:128 directly, so psum->sbuf
    OT copies are same-partition CASTs (no stream shuffle).
  - Even row-sum rows parked in an SBUF tile via 1-row ACT copies and
    gathered by K=1 matmuls at sigma end; reciprocal_approx_fast (DVE)
    overlaps the next sigma's scores/AV.
  - The PE instruction stream is software-pipelined across phases: only
    QK chunks m0/m1 + V(src0) run as an upfront block per batch; QK
    m2-5 + V(src1), the previous sigma's normalize+out-proj, and the
    next batch's x transposes are all interleaved into the scores/AV hh
    loops.  This keeps the PE dense through the ACT-exp latency and the
    softmax-denominator barrier, holding the HAM clock gate at K=8/8
    (the unpipelined fp32r version lost ~55% of its runtime to K=4/8
    throttling; this version loses one 3.4us HAM window per batch).
  - PSUM: 6-slot bank ring (tag "sm") + 2 rsum banks.
"""

import numpy as np

import concourse.bass as bass
import concourse.bacc as bacc
import concourse.mybir as mybir
import concourse.tile as tile
from concourse.bass_utils import run_bass_kernel_spmd
from concourse.masks import make_identity

FP32 = mybir.dt.float32
FP32R = mybir.dt.float32r
BF16 = mybir.dt.bfloat16
AF = mybir.ActivationFunctionType

H, DH, S, C = 12, 64, 197, 768
NCO = C // 128  # 6 channel chunks
SCALE = DH ** -0.5
S_TILES = [(0, 128), (128, 69)]
N_CHUNKS = [(0, 384), (384, 384)]  # out-proj/V-proj column chunks (6 heads)
# (key/value source, query source) -> output stream index; 0=base, 1=target
STREAM_IDX = {(0, 0): 0, (0, 1): 3, (1, 1): 1, (1, 0): 2}
N_CORES = 8
S2 = 2 * S  # query axis covers both query sources side by side


def _ap(base, free_dims):
    """AP with base's partition dim and explicit free [stride, size] dims."""
    return bass.AP(tensor=base.tensor, offset=base.offset,
                   ap=[list(base.ap[0])] + [list(d) for d in free_dims])


def build_nc(B_L):
    nc = bacc.Bacc("TRN2", target_bir_lowering=False, debug=False,
                   num_devices=N_CORES)

    x_in = {
        0: nc.dram_tensor("x_base", [B_L, S, C], FP32, kind="ExternalInput"),
        1: nc.dram_tensor("x_target", [B_L, S, C], FP32, kind="ExternalInput"),
    }
    w_dram, b_dram = {}, {}
    for nm in ("q", "k", "v", "p"):
        w_dram[nm] = nc.dram_tensor(f"W{nm}", [C, C], FP32, kind="ExternalInput")
        b_dram[nm] = nc.dram_tensor(f"b{nm}", [C], FP32, kind="ExternalInput")
    out_d = nc.dram_tensor("out", [4, B_L, S, C], FP32, kind="ExternalOutput")

    with tile.TileContext(nc) as tc:
        with (
            tc.tile_pool(name="const", bufs=1) as constp,
            tc.tile_pool(name="stage", bufs=6) as stagep,
            tc.tile_pool(name="wsb", bufs=1) as wp,
            tc.tile_pool(name="xt", bufs=2) as xtp,
            tc.tile_pool(name="qkv", bufs=2) as qkvp,
            tc.tile_pool(name="expp", bufs=4) as expp,
            tc.tile_pool(name="ot", bufs=2) as otp,
            tc.tile_pool(name="rpool", bufs=2) as rp,
            tc.tile_pool(name="y2", bufs=3) as y2p,
            tc.tile_pool(name="ps", bufs=6, space="PSUM") as ps,
            tc.tile_pool(name="psr", bufs=2, space="PSUM") as psr,
        ):
            # ---- constants ----
            ident = constp.tile([128, 128], FP32)
            make_identity(nc, ident)

            # E[h, c] = 1 iff channel c belongs to head h (normalize bcast)
            E_f32 = constp.tile([H, C], FP32)
            nc.gpsimd.memset(E_f32, 1.0)
            nc.gpsimd.affine_select(
                out=E_f32, in_=E_f32, compare_op=mybir.AluOpType.is_ge, fill=0.0,
                base=0, pattern=[[1, C]], channel_multiplier=-DH)
            nc.gpsimd.affine_select(
                out=E_f32, in_=E_f32, compare_op=mybir.AluOpType.is_ge, fill=0.0,
                base=DH - 1, pattern=[[-1, C]], channel_multiplier=DH)
            E_sb = constp.tile([H, C], FP32R)
            nc.vector.tensor_copy(out=E_sb, in_=E_f32)

            # E3[p, h, j] = (j == h): one-hot rows used (at partitions 63/64)
            # to gather each head's AV rowsum row into one [H, S2] psum
            E3_f32 = constp.tile([128, H, H], FP32)
            nc.gpsimd.memset(E3_f32, 0.0)
            for h in range(H):
                nc.gpsimd.memset(E3_f32[:, h, h:h + 1], 1.0)
            E3_sb = constp.tile([128, H, H], BF16)
            nc.vector.tensor_copy(out=E3_sb, in_=E3_f32)

            # ones_sel[p, i, j] = (j == 2i+1): all-ones column per odd head,
            # used as matmul lhsT to reduce exp over keys -> rowsum row 2i+1
            # of the rsum psum tile (odd heads carry no ones column in V).
            osel_f32 = constp.tile([128, H // 2, H], FP32)
            nc.gpsimd.memset(osel_f32, 0.0)
            for i in range(H // 2):
                nc.gpsimd.memset(osel_f32[:, i, 2 * i + 1:2 * i + 2], 1.0)
            ones_sel = constp.tile([128, H // 2, H], BF16)
            nc.vector.tensor_copy(out=ones_sel, in_=osel_f32)

            # fp32 ones used to write the bf16 ones-columns of V via DVE copy
            ones_c = constp.tile([128, H], FP32)
            nc.gpsimd.memset(ones_c, 1.0)

            # per-partition channel biases for the transposed Q/K outputs
            bqk_sb = {}
            for nm in ("q", "k"):
                t = constp.tile([128, NCO], FP32, name=f"b{nm}_sb")
                nc.gpsimd.dma_start(
                    out=t, in_=b_dram[nm].rearrange("(ko p) -> p ko", p=128))
                bqk_sb[nm] = t
            # biases broadcast along partitions for natural-layout outputs
            bbc_sb = {}
            for nm in ("v", "p"):
                t = constp.tile([128, C], FP32, name=f"b{nm}_bc")
                src_ap = b_dram[nm][:]
                bcast = bass.AP(tensor=src_ap.tensor, offset=src_ap.offset,
                                ap=[[0, 128]] + list(src_ap.ap))
                nc.gpsimd.dma_start(out=t, in_=bcast)
                bbc_sb[nm] = t

            # ---- prefetch batch-0 x tiles ahead of the weight loads so the
            # PE transposes can start immediately ----
            x_stage = {}
            for src in (0, 1):
                for (s0, s_sz) in S_TILES:
                    xs = stagep.tile([128, C], FP32, tag="stage", name="xs")
                    nc.sync.dma_start(out=xs[:s_sz, :],
                                      in_=x_in[src][0, s0:s0 + s_sz, :])
                    x_stage[(0, src, s0)] = xs

            # ---- weights: DMA fp32 then DVE-cast to bf16 ----
            W_sb = {}
            for nm in ("q", "k", "v", "p"):
                W_sb[nm] = wp.tile([128, NCO, C], BF16, tag=f"w{nm}",
                                   name=f"W{nm}_sb")
                for ko in range(NCO):
                    st = stagep.tile([128, C], FP32, tag="wstage", name="wst",
                                     bufs=3)
                    nc.sync.dma_start(out=st,
                                      in_=w_dram[nm][ko * 128:(ko + 1) * 128, :])
                    nc.vector.tensor_copy(out=W_sb[nm][:, ko, :], in_=st)

            def make_tail(tb, tsigma, OT_raw_p, rr32_p):
                """Normalize + out-proj for a finished sigma, sliced into
                H//2 piece-lists for interleaving into the next sigma's
                scores/AV loop."""
                OT = otp.tile([128, NCO, S2], BF16, tag="ot")
                ycur = [None]

                def pr_piece(co):
                    pr = ps.tile([128, S2], FP32, tag="sm", name="pr")
                    nc.tensor.matmul(
                        pr[:], lhsT=E_sb[:, co * 128:(co + 1) * 128],
                        rhs=rr32_p[:], start=True, stop=True)
                    nc.vector.tensor_mul(out=OT[:, co, :],
                                         in0=OT_raw_p[:, co, :], in1=pr[:])

                def py_piece(g):
                    qs, sti_, nci = g // 4, (g // 2) % 2, g % 2
                    s0, s_sz = S_TILES[sti_]
                    n0, n_sz = N_CHUNKS[nci]
                    if nci == 0:
                        ycur[0] = y2p.tile([128, C], FP32, tag="y2",
                                           name="y")
                    y = ycur[0]
                    py = ps.tile([128, 384], FP32, tag="sm", name="py")
                    for k in range(NCO):
                        nc.tensor.matmul(
                            py[:s_sz, :n_sz],
                            lhsT=OT[:, k, qs * S + s0:qs * S + s0 + s_sz],
                            rhs=W_sb["p"][:, k, n0:n0 + n_sz],
                            start=(k == 0), stop=(k == NCO - 1))
                    nc.vector.tensor_add(
                        out=y[:s_sz, n0:n0 + n_sz], in0=py[:s_sz, :n_sz],
                        in1=bbc_sb["p"][:s_sz, n0:n0 + n_sz])
                    if nci == 1:
                        stream = STREAM_IDX[(tsigma, qs)]
                        nc.sync.dma_start(
                            out=out_d[stream, tb, s0:s0 + s_sz, :],
                            in_=y[:s_sz, :])

                def P(fn, *a):
                    return lambda: fn(*a)

                return [[P(pr_piece, 0), P(pr_piece, 1), P(pr_piece, 2)],
                        [P(pr_piece, 3), P(pr_piece, 4), P(pr_piece, 5),
                         P(py_piece, 0)],
                        [P(py_piece, 1), P(py_piece, 2)],
                        [P(py_piece, 3), P(py_piece, 4)],
                        [P(py_piece, 5)],
                        [P(py_piece, 6)],
                        [P(py_piece, 7)]]

            def emit_transpose(XT, tb, src, sti):
                s0, s_sz = S_TILES[sti]
                xs = x_stage.pop((tb, src, s0))
                for co in range(NCO):
                    pt = ps.tile([128, 128], FP32, tag="sm", name="pt")
                    nc.tensor.transpose(
                        pt[:, :s_sz], xs[:s_sz, co * 128:(co + 1) * 128],
                        ident[:s_sz, :s_sz])
                    nc.vector.tensor_copy(
                        out=XT[:, co, src, s0:s0 + s_sz], in_=pt[:, :s_sz])

            def make_transpose_pieces(tb):
                """Next batch's x transposes, sliced for interleaving into
                the current sigma's hh loop (transposes are HAM-invisible;
                standalone they demote the PE clock every batch)."""
                XT = xtp.tile([128, NCO, 2, S], BF16, tag="xt")

                def tp(src, sti):
                    emit_transpose(XT, tb, src, sti)

                def P(fn, *a):
                    return lambda: fn(*a)

                return XT, [[], [P(tp, 0, 0)], [P(tp, 0, 1)],
                            [P(tp, 1, 0)], [P(tp, 1, 1)], [], []]

            pending = None
            qkv_next = None

            # batch-0 transposes (prologue; x already prefetched)
            XT = xtp.tile([128, NCO, 2, S], BF16, tag="xt")
            for src in (0, 1):
                for sti in range(len(S_TILES)):
                    emit_transpose(XT, 0, src, sti)

            for b in range(B_L):
                # prefetch next batch's x tiles (overlaps this batch)
                if b + 1 < B_L:
                    for src in (0, 1):
                        for (s0, s_sz) in S_TILES:
                            xs = stagep.tile([128, C], FP32, tag="stage",
                                             name="xs")
                            nc.sync.dma_start(
                                out=xs[:s_sz, :],
                                in_=x_in[src][b + 1, s0:s0 + s_sz, :])
                            x_stage[(b + 1, src, s0)] = xs

                # ---- Q/K/V projections.  Only QK chunks m0/m1 and V(src0)
                # run as an upfront block (emitted during the previous
                # batch's sigma-1 loop for b>0); QK m2-5 and V(src1) are
                # interleaved into sigma-0's scores/AV loop below. ----
                if qkv_next is not None:
                    QT, KT, V_sb = qkv_next
                    qkv_next = None
                else:
                    QT = qkvp.tile([128, NCO, 2, S], BF16, tag="qt")
                    KT = qkvp.tile([128, NCO, 2, S], BF16, tag="kt")
                    V_sb = qkvp.tile([128, 2, 2, H, DH + 1], BF16, tag="v")
                XT_b = XT

                def emit_qk(nm, m, XTl=None, QTl=None, KTl=None):
                    XTl = XT_b if XTl is None else XTl
                    OUT = (QT if QTl is None else QTl) if nm == "q" else \
                          (KT if KTl is None else KTl)
                    pp = [ps.tile([128, S], FP32, tag="sm", name="pp0"),
                          ps.tile([128, S], FP32, tag="sm", name="pp1")]
                    for k in range(NCO):
                        for src in (0, 1):
                            nc.tensor.matmul(
                                pp[src][:],
                                lhsT=W_sb[nm][:, k, m * 128:(m + 1) * 128],
                                rhs=XTl[:, k, src, :],
                                start=(k == 0), stop=(k == NCO - 1))
                    for src in (0, 1):
                        nc.scalar.activation(
                            out=OUT[:, m, src, :], in_=pp[src][:],
                            func=AF.Identity,
                            bias=bqk_sb[nm][:, m:m + 1], scale=1.0)

                def emit_v(src, sti, nci, XTl=None, Vl=None):
                    XTl = XT_b if XTl is None else XTl
                    Vl = V_sb if Vl is None else Vl
                    s0, s_sz = S_TILES[sti]
                    n0, n_sz = N_CHUNKS[nci]
                    pv = ps.tile([128, 384], FP32, tag="sm", name="pv")
                    for k in range(NCO):
                        nc.tensor.matmul(
                            pv[:s_sz, :n_sz],
                            lhsT=XTl[:, k, src, s0:s0 + s_sz],
                            rhs=W_sb["v"][:, k, n0:n0 + n_sz],
                            start=(k == 0), stop=(k == NCO - 1))
                    h0, nh = n0 // DH, n_sz // DH
                    nc.vector.tensor_add(
                        out=Vl[:s_sz, src, sti, h0:h0 + nh, 0:DH],
                        in0=pv[:s_sz, :n_sz].rearrange(
                            "p (h d) -> p h d", d=DH),
                        in1=bbc_sb["v"][:s_sz, n0:n0 + n_sz].rearrange(
                            "p (h d) -> p h d", d=DH))
                    if nci == len(N_CHUNKS) - 1:
                        dst_e = _ap(Vl[0:s_sz, src, sti, 0, DH],
                                    [[2 * (DH + 1), H // 2]])
                        nc.vector.tensor_copy(out=dst_e,
                                              in_=ones_c[:s_sz, :H // 2])

                if b == 0:
                    for m in (0, 1):
                        for nm in ("q", "k"):
                            emit_qk(nm, m)
                    for sti in range(len(S_TILES)):
                        for nci in range(len(N_CHUNKS)):
                            emit_v(0, sti, nci)

                # ---- attention.  scores+AV+rowsum per sigma, with the
                # PREVIOUS sigma's normalize+out-proj matmuls interleaved
                # into the hh loop: the out-proj work fills the PE bubbles
                # left by ACT exp latency (keeps the HAM clock gate at 8/8),
                # and the softmax-denominator reciprocal gets a whole
                # scores/AV phase to complete off the critical path. ----
                for sigma in (0, 1):
                    OT_raw = otp.tile([128, NCO, S2], BF16, tag="otraw")
                    rsball = rp.tile([DH + 1, H, S2], BF16, tag="rsb")
                    rsum = psr.tile([128, S2], FP32, tag="rsum")
                    ets = {}

                    def emit_scores(hh):
                        for sti in range(len(S_TILES)):
                            ets[(hh, sti)] = expp.tile([128, 2, S2], BF16,
                                                       tag="exp", name="et")
                        # j-major so AV j=0's two exp deps complete first
                        for j in (0, 1):
                            for sti, (s0, s_sz) in enumerate(S_TILES):
                                et = ets[(hh, sti)]
                                psc = ps.tile([128, S2], FP32, tag="sm",
                                              name="psc")
                                nc.tensor.matmul(
                                    psc[:s_sz, :],
                                    lhsT=KT[j * DH:(j + 1) * DH, hh, sigma,
                                            s0:s0 + s_sz],
                                    rhs=QT[j * DH:(j + 1) * DH, hh, :, :],
                                    start=True, stop=True)
                                nc.scalar.activation(
                                    out=et[:s_sz, j, :], in_=psc[:s_sz, :],
                                    func=AF.Exp, scale=float(SCALE))

                    def emit_av(hh):
                        et = [ets.pop((hh, 0)), ets.pop((hh, 1))]
                        for j in (0, 1):
                            h = 2 * hh + j
                            pav = ps.tile([128, S2], FP32, tag="sm",
                                          name="pav")
                            if j == 0:
                                # even: ones col in V -> rowsum at row DH
                                for sti, (s0, s_sz) in enumerate(S_TILES):
                                    nc.tensor.matmul(
                                        pav[0:DH + 1, :],
                                        lhsT=V_sb[:s_sz, sigma, sti, h, :],
                                        rhs=et[sti][:s_sz, j, :],
                                        start=(sti == 0), stop=(sti == 1))
                                nc.scalar.copy(out=rsball[DH:DH + 1, h, :],
                                               in_=pav[DH:DH + 1, :])
                                nc.vector.tensor_copy(
                                    out=OT_raw[0:DH, hh, :],
                                    in_=pav[0:DH, :])
                            else:
                                # odd: data straight to partitions 64:128;
                                # rowsum via ones_sel into the rsum accum
                                for sti, (s0, s_sz) in enumerate(S_TILES):
                                    nc.tensor.matmul(
                                        pav[DH:2 * DH, :],
                                        lhsT=V_sb[:s_sz, sigma, sti, h, 0:DH],
                                        rhs=et[sti][:s_sz, j, :],
                                        start=(sti == 0), stop=(sti == 1))
                                for sti, (s0, s_sz) in enumerate(S_TILES):
                                    nc.tensor.matmul(
                                        rsum[:H, :],
                                        lhsT=ones_sel[:s_sz, hh, :],
                                        rhs=et[sti][:s_sz, j, :],
                                        start=(hh == 0 and sti == 0),
                                        stop=False)
                                nc.vector.tensor_copy(
                                    out=OT_raw[DH:2 * DH, hh, :],
                                    in_=pav[DH:2 * DH, :])

                    pieces = (make_tail(*pending) if pending is not None
                              else [[] for _ in range(H // 2 + 1)])
                    if sigma == 0:
                        def QP(nm, m):
                            return lambda: emit_qk(nm, m)

                        def VP(sti, nci):
                            return lambda: emit_v(1, sti, nci)

                        for i, m in enumerate((2, 3, 4, 5)):
                            pieces[i] = [QP("q", m), QP("k", m)] + pieces[i]
                        pieces[2].append(VP(0, 0))
                        pieces[3].append(VP(1, 0))
                        pieces[4].append(VP(0, 1))
                        pieces[6].append(VP(1, 1))
                    elif b + 1 < B_L:
                        XT_next, tpieces = make_transpose_pieces(b + 1)
                        pieces = [a + t for a, t in zip(pieces, tpieces)]
                        QT_n = qkvp.tile([128, NCO, 2, S], BF16, tag="qt",
                                         name="QT_n")
                        KT_n = qkvp.tile([128, NCO, 2, S], BF16, tag="kt",
                                         name="KT_n")
                        V_n = qkvp.tile([128, 2, 2, H, DH + 1], BF16,
                                        tag="v", name="V_n")
                        qkv_next = (QT_n, KT_n, V_n)

                        def NQ(nm, m):
                            return lambda: emit_qk(nm, m, XT_next, QT_n, KT_n)

                        def NV(sti, nci):
                            return lambda: emit_v(0, sti, nci, XT_next, V_n)

                        pieces[5] += [NQ("q", 0), NQ("k", 0), NV(0, 0)]
                        pieces[6] += [NQ("q", 1), NQ("k", 1), NV(1, 0),
                                      NV(0, 1), NV(1, 1)]
                    for hh in range(H // 2):
                        emit_scores(hh)
                        for fn in pieces[hh]:
                            fn()
                        if hh:
                            emit_av(hh - 1)
                    for fn in pieces[H // 2]:
                        fn()
                    emit_av(H // 2 - 1)

                    # gather the 6 even rowsum rows -> rsum[0:H] (K=1)
                    for h in range(0, H, 2):
                        nc.tensor.matmul(
                            rsum[:H, :], lhsT=E3_sb[DH:DH + 1, h, :],
                            rhs=rsball[DH:DH + 1, h, :],
                            start=False, stop=(h == H - 2))
                    rr32 = rp.tile([H, S2], FP32, tag="rrf")
                    nc.vector.reciprocal_approx_fast(out=rr32,
                                                     in_=rsum[:H, :])
                    rr = rp.tile([H, S2], FP32R, tag="rr")
                    with nc.allow_low_precision(reason="softmax denom"):
                        nc.vector.tensor_copy(out=rr, in_=rr32)
                    pending = (b, sigma, OT_raw, rr)

                if b + 1 < B_L:
                    XT = XT_next

            # last sigma's normalize+out-proj (nothing left to interleave)
            for piece in make_tail(*pending):
                for fn in piece:
                    fn()
    nc.compile()
    return nc


_NC_CACHE = {}


def _get_nc(B_L):
    if B_L not in _NC_CACHE:
        _NC_CACHE[B_L] = build_nc(B_L)
    return _NC_CACHE[B_L]


def _make_in_maps(inputs):
    inputs = {k: np.ascontiguousarray(np.asarray(v), dtype=np.float32)
              for k, v in inputs.items()}
    B = inputs["x_base"].shape[0]
    assert B % N_CORES == 0, f"batch {B} not divisible by {N_CORES} cores"
    B_L = B // N_CORES
    shared = {k: inputs[k] for k in
              ("Wq", "bq", "Wk", "bk", "Wv", "bv", "Wp", "bp")}
    in_maps = []
    for i in range(N_CORES):
        m = dict(shared)
        m["x_base"] = np.ascontiguousarray(inputs["x_base"][i * B_L:(i + 1) * B_L])
        m["x_target"] = np.ascontiguousarray(inputs["x_target"][i * B_L:(i + 1) * B_L])
        in_maps.append(m)
    return in_maps


def kernel(**inputs):
    B = np.asarray(inputs["x_base"]).shape[0]
    B_L = B // N_CORES
    nc = _get_nc(B_L)
    in_maps = _make_in_maps(inputs)
    res = run_bass_kernel_spmd(nc, in_maps, core_ids=list(range(N_CORES)))
    return np.concatenate([r["out"] for r in res.results], axis=1)


# revision 33
# speedup vs baseline: 1.0035x; 1.0035x over previous
"""Trainium2 Bass kernel for nn_Attention_86217173500445.

Cross-attention block: shared QKV projections over two inputs (base/target),
4 attention streams (bb, tt, bt, tb), shared output projection.

Strategy: data-parallel over batch (B=32 -> 4 per core on 8 cores), weights
replicated, zero collectives.  Per-core compute is a fused bf16 pipeline
(fp32 PSUM accumulation everywhere, tolerance is 2e-2):
  - x transposed on-chip (PE transpose, fp32) to XT [C, S] bf16.
  - Q/K projections emit transposed QT/KT [C, 2src, S] bf16; V natural
    layout with a ones column (col 64) for even heads only, so their AV
    matmuls produce softmax row-sums for free; odd heads' row-sums come
    from ones_sel (all-ones lhsT column) matmuls accumulating straight
    into the rsum PSUM tile.
  - ScoresT [k, q] per (head, j), j-major; ACT exp -> bf16 feeds AV.
  - Odd heads' AV writes PSUM partitions 64:128 directly, so psum->sbuf
    OT copies are same-partition CASTs (no stream shuffle).
  - Even row-sum rows parked in an SBUF tile via 1-row ACT copies and
    gathered by K=1 matmuls at sigma end; reciprocal_approx_fast (DVE)
    overlaps the next sigma's scores/AV.
  - The PE instruction stream is software-pipelined across phases: only
    QK chunks m0/m1 + V(src0) run as an upfront block per batch; QK
    m2-5 + V(src1), the previous sigma's normalize+out-proj, and the
    next batch's x transposes are all interleaved into the scores/AV hh
    loops.  This keeps the PE dense through the ACT-exp latency and the
    softmax-denominator barrier, holding the HAM clock gate at K=8/8
    (the unpipelined fp32r version lost ~55% of its runtime to K=4/8
    throttling; this version loses one 3.4us HAM window per batch).
  - PSUM: 6-slot bank ring (tag "sm") + 2 rsum banks.
"""

import numpy as np

import concourse.bass as bass
import concourse.bacc as bacc
import concourse.mybir as mybir
import concourse.tile as tile
from concourse.bass_utils import run_bass_kernel_spmd
from concourse.masks import make_identity

FP32 = mybir.dt.float32
FP32R = mybir.dt.float32r
BF16 = mybir.dt.bfloat16
AF = mybir.ActivationFunctionType

H, DH, S, C = 12, 64, 197, 768
NCO = C // 128  # 6 channel chunks
SCALE = DH ** -0.5
S_TILES = [(0, 128), (128, 69)]
N_CHUNKS = [(0, 384), (384, 384)]  # out-proj/V-proj column chunks (6 heads)
# (key/value source, query source) -> output stream index; 0=base, 1=target
STREAM_IDX = {(0, 0): 0, (0, 1): 3, (1, 1): 1, (1, 0): 2}
N_CORES = 8
S2 = 2 * S  # query axis covers both query sources side by side


def _ap(base, free_dims):
    """AP with base's partition dim and explicit free [stride, size] dims."""
    return bass.AP(tensor=base.tensor, offset=base.offset,
                   ap=[list(base.ap[0])] + [list(d) for d in free_dims])


def build_nc(B_L):
    nc = bacc.Bacc("TRN2", target_bir_lowering=False, debug=False,
                   num_devices=N_CORES)

    x_in = {
        0: nc.dram_tensor("x_base", [B_L, S, C], FP32, kind="ExternalInput"),
        1: nc.dram_tensor("x_target", [B_L, S, C], FP32, kind="ExternalInput"),
    }
    w_dram, b_dram = {}, {}
    for nm in ("q", "k", "v", "p"):
        w_dram[nm] = nc.dram_tensor(f"W{nm}", [C, C], FP32, kind="ExternalInput")
        b_dram[nm] = nc.dram_tensor(f"b{nm}", [C], FP32, kind="ExternalInput")
    out_d = nc.dram_tensor("out", [4, B_L, S, C], FP32, kind="ExternalOutput")

    with tile.TileContext(nc) as tc:
        with (
            tc.tile_pool(name="const", bufs=1) as constp,
            tc.tile_pool(name="stage", bufs=6) as stagep,
            tc.tile_pool(name="wsb", bufs=1) as wp,
            tc.tile_pool(name="xt", bufs=2) as xtp,
            tc.tile_pool(name="qkv", bufs=2) as qkvp,
            tc.tile_pool(name="expp", bufs=4) as expp,
            tc.tile_pool(name="ot", bufs=2) as otp,
            tc.tile_pool(name="rpool", bufs=2) as rp,
            tc.tile_pool(name="y2", bufs=3) as y2p,
            tc.tile_pool(name="ps", bufs=6, space="PSUM") as ps,
            tc.tile_pool(name="psr", bufs=2, space="PSUM") as psr,
        ):
            # ---- constants ----
            ident = constp.tile([128, 128], FP32)
            make_identity(nc, ident)

            # E[h, c] = 1 iff channel c belongs to head h (normalize bcast)
            E_f32 = constp.tile([H, C], FP32)
            nc.gpsimd.memset(E_f32, 1.0)
            nc.gpsimd.affine_select(
                out=E_f32, in_=E_f32, compare_op=mybir.AluOpType.is_ge, fill=0.0,
                base=0, pattern=[[1, C]], channel_multiplier=-DH)
            nc.gpsimd.affine_select(
                out=E_f32, in_=E_f32, compare_op=mybir.AluOpType.is_ge, fill=0.0,
                base=DH - 1, pattern=[[-1, C]], channel_multiplier=DH)
            E_sb = constp.tile([H, C], FP32R)
            nc.vector.tensor_copy(out=E_sb, in_=E_f32)

            # E3[p, h, j] = (j == h): one-hot rows used (at partitions 63/64)
            # to gather each head's AV rowsum row into one [H, S2] psum
            E3_f32 = constp.tile([128, H, H], FP32)
            nc.gpsimd.memset(E3_f32, 0.0)
            for h in range(H):
                nc.gpsimd.memset(E3_f32[:, h, h:h + 1], 1.0)
            E3_sb = constp.tile([128, H, H], BF16)
            nc.vector.tensor_copy(out=E3_sb, in_=E3_f32)

            # ones_sel[p, i, j] = (j == 2i+1): all-ones column per odd head,
            # used as matmul lhsT to reduce exp over keys -> rowsum row 2i+1
            # of the rsum psum tile (odd heads carry no ones column in V).
            osel_f32 = constp.tile([128, H // 2, H], FP32)
            nc.gpsimd.memset(osel_f32, 0.0)
            for i in range(H // 2):
                nc.gpsimd.memset(osel_f32[:, i, 2 * i + 1:2 * i + 2], 1.0)
            ones_sel = constp.tile([128, H // 2, H], BF16)
            nc.vector.tensor_copy(out=ones_sel, in_=osel_f32)

            # fp32 ones used to write the bf16 ones-columns of V via DVE copy
            ones_c = constp.tile([128, H], FP32)
            nc.gpsimd.memset(ones_c, 1.0)

            # per-partition channel biases for the transposed Q/K outputs
            bqk_sb = {}
            for nm in ("q", "k"):
                t = constp.tile([128, NCO], FP32, name=f"b{nm}_sb")
                nc.gpsimd.dma_start(
                    out=t, in_=b_dram[nm].rearrange("(ko p) -> p ko", p=128))
                bqk_sb[nm] = t
            # biases broadcast along partitions for natural-layout outputs
            bbc_sb = {}
            for nm in ("v", "p"):
                t = constp.tile([128, C], FP32, name=f"b{nm}_bc")
                src_ap = b_dram[nm][:]
                bcast = bass.AP(tensor=src_ap.tensor, offset=src_ap.offset,
                                ap=[[0, 128]] + list(src_ap.ap))
                nc.gpsimd.dma_start(out=t, in_=bcast)
                bbc_sb[nm] = t

            # ---- prefetch batch-0 x tiles ahead of the weight loads so the
            # PE transposes can start immediately ----
            x_stage = {}
            for src in (0, 1):
                for (s0, s_sz) in S_TILES:
                    xs = stagep.tile([128, C], FP32, tag="stage", name="xs")
                    nc.sync.dma_start(out=xs[:s_sz, :],
                                      in_=x_in[src][0, s0:s0 + s_sz, :])
                    x_stage[(0, src, s0)] = xs

            # ---- weights: DMA fp32 then DVE-cast to bf16 ----
            W_sb = {}
            for nm in ("q", "k", "v", "p"):
                W_sb[nm] = wp.tile([128, NCO, C], BF16, tag=f"w{nm}",
                                   name=f"W{nm}_sb")
                for ko in range(NCO):
                    st = stagep.tile([128, C], FP32, tag="wstage", name="wst",
                                     bufs=3)
                    nc.sync.dma_start(out=st,
                                      in_=w_dram[nm][ko * 128:(ko + 1) * 128, :])
                    nc.vector.tensor_copy(out=W_sb[nm][:, ko, :], in_=st)

            def make_tail(tb, tsigma, OT_raw_p, rr32_p):
                """Normalize + out-proj for a finished sigma, sliced into
                H//2 piece-lists for interleaving into the next sigma's
                scores/AV loop."""
                OT = otp.tile([128, NCO, S2], BF16, tag="ot")
                ycur = [None]

                def pr_piece(co):
                    pr = ps.tile([128, S2], FP32, tag="sm", name="pr")
                    nc.tensor.matmul(
                        pr[:], lhsT=E_sb[:, co * 128:(co + 1) * 128],
                        rhs=rr32_p[:], start=True, stop=True)
                    nc.vector.tensor_mul(out=OT[:, co, :],
                                         in0=OT_raw_p[:, co, :], in1=pr[:])

                def py_piece(g):
                    qs, sti_, nci = g // 4, (g // 2) % 2, g % 2
                    s0, s_sz = S_TILES[sti_]
                    n0, n_sz = N_CHUNKS[nci]
                    if nci == 0:
                        ycur[0] = y2p.tile([128, C], FP32, tag="y2",
                                           name="y")
                    y = ycur[0]
                    py = ps.tile([128, 384], FP32, tag="sm", name="py")
                    for k in range(NCO):
                        nc.tensor.matmul(
                            py[:s_sz, :n_sz],
                            lhsT=OT[:, k, qs * S + s0:qs * S + s0 + s_sz],
                            rhs=W_sb["p"][:, k, n0:n0 + n_sz],
                            start=(k == 0), stop=(k == NCO - 1))
                    nc.vector.tensor_add(
                        out=y[:s_sz, n0:n0 + n_sz], in0=py[:s_sz, :n_sz],
                        in1=bbc_sb["p"][:s_sz, n0:n0 + n_sz])
                    if nci == 1:
                        stream = STREAM_IDX[(tsigma, qs)]
                        nc.sync.dma_start(
                            out=out_d[stream, tb, s0:s0 + s_sz, :],
                            in_=y[:s_sz, :])

                def P(fn, *a):
                    return lambda: fn(*a)

                return [[P(pr_piece, 0), P(pr_piece, 1), P(pr_piece, 2)],
                        [P(pr_piece, 3), P(pr_piece, 4), P(pr_piece, 5),
                         P(py_piece, 0)],
                        [P(py_piece, 1), P(py_piece, 2)],
                        [P(py_piece, 3), P(py_piece, 4)],
                        [P(py_piece, 5)],
                        [P(py_piece, 6)],
                        [P(py_piece, 7)]]

            def emit_transpose(XT, tb, src, sti):
                s0, s_sz = S_TILES[sti]
                xs = x_stage.pop((tb, src, s0))
                for co in range(NCO):
                    pt = ps.tile([128, 128], FP32, tag="sm", name="pt")
                    nc.tensor.transpose(
                        pt[:, :s_sz], xs[:s_sz, co * 128:(co + 1) * 128],
                        ident[:s_sz, :s_sz])
                    nc.vector.tensor_copy(
                        out=XT[:, co, src, s0:s0 + s_sz], in_=pt[:, :s_sz])

            def make_transpose_pieces(tb):
                """Next batch's x transposes, sliced for interleaving into
                the current sigma's hh loop (transposes are HAM-invisible;
                standalone they demote the PE clock every batch)."""
                XT = xtp.tile([128, NCO, 2, S], BF16, tag="xt")

                def tp(src, sti):
                    emit_transpose(XT, tb, src, sti)

                def P(fn, *a):
                    return lambda: fn(*a)

                return XT, [[], [P(tp, 0, 0)], [P(tp, 0, 1)],
                            [P(tp, 1, 0)], [P(tp, 1, 1)], [], []]

            pending = None

            # batch-0 transposes (prologue; x already prefetched)
            XT = xtp.tile([128, NCO, 2, S], BF16, tag="xt")
            for src in (0, 1):
                for sti in range(len(S_TILES)):
                    emit_transpose(XT, 0, src, sti)

            for b in range(B_L):
                # prefetch next batch's x tiles (overlaps this batch)
                if b + 1 < B_L:
                    for src in (0, 1):
                        for (s0, s_sz) in S_TILES:
                            xs = stagep.tile([128, C], FP32, tag="stage",
                                             name="xs")
                            nc.sync.dma_start(
                                out=xs[:s_sz, :],
                                in_=x_in[src][b + 1, s0:s0 + s_sz, :])
                            x_stage[(b + 1, src, s0)] = xs

                # ---- Q/K/V projections.  Only QK chunks m0/m1 and V(src0)
                # are emitted as an upfront block; QK m2-5 and V(src1) are
                # interleaved into sigma-0's scores/AV loop below. ----
                QT = qkvp.tile([128, NCO, 2, S], BF16, tag="qt")
                KT = qkvp.tile([128, NCO, 2, S], BF16, tag="kt")
                V_sb = qkvp.tile([128, 2, 2, H, DH + 1], BF16, tag="v")
                XT_b = XT

                def emit_qk(nm, m, XTl=None, QTl=None, KTl=None):
                    XTl = XT_b if XTl is None else XTl
                    OUT = (QT if QTl is None else QTl) if nm == "q" else \
                          (KT if KTl is None else KTl)
                    pp = [ps.tile([128, S], FP32, tag="sm", name="pp0"),
                          ps.tile([128, S], FP32, tag="sm", name="pp1")]
                    for k in range(NCO):
                        for src in (0, 1):
                            nc.tensor.matmul(
                                pp[src][:],
                                lhsT=W_sb[nm][:, k, m * 128:(m + 1) * 128],
                                rhs=XTl[:, k, src, :],
                                start=(k == 0), stop=(k == NCO - 1))
                    for src in (0, 1):
                        nc.scalar.activation(
                            out=OUT[:, m, src, :], in_=pp[src][:],
                            func=AF.Identity,
                            bias=bqk_sb[nm][:, m:m + 1], scale=1.0)

                def emit_v(src, sti, nci):
                    s0, s_sz = S_TILES[sti]
                    n0, n_sz = N_CHUNKS[nci]
                    pv = ps.tile([128, 384], FP32, tag="sm", name="pv")
                    for k in range(NCO):
                        nc.tensor.matmul(
                            pv[:s_sz, :n_sz],
                            lhsT=XT_b[:, k, src, s0:s0 + s_sz],
                            rhs=W_sb["v"][:, k, n0:n0 + n_sz],
                            start=(k == 0), stop=(k == NCO - 1))
                    h0, nh = n0 // DH, n_sz // DH
                    nc.vector.tensor_add(
                        out=V_sb[:s_sz, src, sti, h0:h0 + nh, 0:DH],
                        in0=pv[:s_sz, :n_sz].rearrange(
                            "p (h d) -> p h d", d=DH),
                        in1=bbc_sb["v"][:s_sz, n0:n0 + n_sz].rearrange(
                            "p (h d) -> p h d", d=DH))
                    if nci == len(N_CHUNKS) - 1:
                        dst_e = _ap(V_sb[0:s_sz, src, sti, 0, DH],
                                    [[2 * (DH + 1), H // 2]])
                        nc.vector.tensor_copy(out=dst_e,
                                              in_=ones_c[:s_sz, :H // 2])

                for m in (0, 1):
                    for nm in ("q", "k"):
                        emit_qk(nm, m)
                for sti in range(len(S_TILES)):
                    for nci in range(len(N_CHUNKS)):
                        emit_v(0, sti, nci)

                # ---- attention.  scores+AV+rowsum per sigma, with the
                # PREVIOUS sigma's normalize+out-proj matmuls interleaved
                # into the hh loop: the out-proj work fills the PE bubbles
                # left by ACT exp latency (keeps the HAM clock gate at 8/8),
                # and the softmax-denominator reciprocal gets a whole
                # scores/AV phase to complete off the critical path. ----
                for sigma in (0, 1):
                    OT_raw = otp.tile([128, NCO, S2], BF16, tag="otraw")
                    rsball = rp.tile([DH + 1, H, S2], BF16, tag="rsb")
                    rsum = psr.tile([128, S2], FP32, tag="rsum")
                    ets = {}

                    def emit_scores(hh):
                        for sti in range(len(S_TILES)):
                            ets[(hh, sti)] = expp.tile([128, 2, S2], BF16,
                                                       tag="exp", name="et")
                        # j-major so AV j=0's two exp deps complete first
                        for j in (0, 1):
                            for sti, (s0, s_sz) in enumerate(S_TILES):
                                et = ets[(hh, sti)]
                                psc = ps.tile([128, S2], FP32, tag="sm",
                                              name="psc")
                                nc.tensor.matmul(
                                    psc[:s_sz, :],
                                    lhsT=KT[j * DH:(j + 1) * DH, hh, sigma,
                                            s0:s0 + s_sz],
                                    rhs=QT[j * DH:(j + 1) * DH, hh, :, :],
                                    start=True, stop=True)
                                nc.scalar.activation(
                                    out=et[:s_sz, j, :], in_=psc[:s_sz, :],
                                    func=AF.Exp, scale=float(SCALE))

                    def emit_av(hh):
                        et = [ets.pop((hh, 0)), ets.pop((hh, 1))]
                        for j in (0, 1):
                            h = 2 * hh + j
                            pav = ps.tile([128, S2], FP32, tag="sm",
                                          name="pav")
                            if j == 0:
                                # even: ones col in V -> rowsum at row DH
                                for sti, (s0, s_sz) in enumerate(S_TILES):
                                    nc.tensor.matmul(
                                        pav[0:DH + 1, :],
                                        lhsT=V_sb[:s_sz, sigma, sti, h, :],
                                        rhs=et[sti][:s_sz, j, :],
                                        start=(sti == 0), stop=(sti == 1))
                                nc.scalar.copy(out=rsball[DH:DH + 1, h, :],
                                               in_=pav[DH:DH + 1, :])
                                nc.vector.tensor_copy(
                                    out=OT_raw[0:DH, hh, :],
                                    in_=pav[0:DH, :])
                            else:
                                # odd: data straight to partitions 64:128;
                                # rowsum via ones_sel into the rsum accum
                                for sti, (s0, s_sz) in enumerate(S_TILES):
                                    nc.tensor.matmul(
                                        pav[DH:2 * DH, :],
                                        lhsT=V_sb[:s_sz, sigma, sti, h, 0:DH],
                                        rhs=et[sti][:s_sz, j, :],
                                        start=(sti == 0), stop=(sti == 1))
                                for sti, (s0, s_sz) in enumerate(S_TILES):
                                    nc.tensor.matmul(
                                        rsum[:H, :],
                                        lhsT=ones_sel[:s_sz, hh, :],
                                        rhs=et[sti][:s_sz, j, :],
                                        start=(hh == 0 and sti == 0),
                                        stop=False)
                                nc.vector.tensor_copy(
                                    out=OT_raw[DH:2 * DH, hh, :],
                                    in_=pav[DH:2 * DH, :])

                    pieces = (make_tail(*pending) if pending is not None
                              else [[] for _ in range(H // 2 + 1)])
                    if sigma == 0:
                        def QP(nm, m):
                            return lambda: emit_qk(nm, m)

                        def VP(sti, nci):
                            return lambda: emit_v(1, sti, nci)

                        for i, m in enumerate((2, 3, 4, 5)):
                            pieces[i] = [QP("q", m), QP("k", m)] + pieces[i]
                        pieces[2].append(VP(0, 0))
                        pieces[3].append(VP(1, 0))
                        pieces[4].append(VP(0, 1))
                        pieces[6].append(VP(1, 1))
                    elif b + 1 < B_L:
                        XT_next, tpieces = make_transpose_pieces(b + 1)
                        pieces = [a + t for a, t in zip(pieces, tpieces)]
                    for hh in range(H // 2):
                        emit_scores(hh)
                        for fn in pieces[hh]:
                            fn()
                        if hh:
                            emit_av(hh - 1)
                    for fn in pieces[H // 2]:
                        fn()
                    emit_av(H // 2 - 1)

                    # gather the 6 even rowsum rows -> rsum[0:H] (K=1)
                    for h in range(0, H, 2):
                        nc.tensor.matmul(
                            rsum[:H, :], lhsT=E3_sb[DH:DH + 1, h, :],
                            rhs=rsball[DH:DH + 1, h, :],
                            start=False, stop=(h == H - 2))
                    rr32 = rp.tile([H, S2], FP32, tag="rrf")
                    nc.vector.reciprocal_approx_fast(out=rr32,
                                                     in_=rsum[:H, :])
                    rr = rp.tile([H, S2], FP32R, tag="rr")
                    with nc.allow_low_precision(reason="softmax denom"):
                        nc.vector.tensor_copy(out=rr, in_=rr32)
                    pending = (b, sigma, OT_raw, rr)

                if b + 1 < B_L:
                    XT = XT_next

            # last sigma's normalize+out-proj (nothing left to interleave)
            for piece in make_tail(*pending):
                for fn in piece:
                    fn()
    nc.compile()
    return nc


_NC_CACHE = {}


def _get_nc(B_L):
    if B_L not in _NC_CACHE:
        _NC_CACHE[B_L] = build_nc(B_L)
    return _NC_CACHE[B_L]


def _make_in_maps(inputs):
    inputs = {k: np.ascontiguousarray(np.asarray(v), dtype=np.float32)
              for k, v in inputs.items()}
    B = inputs["x_base"].shape[0]
    assert B % N_CORES == 0, f"batch {B} not divisible by {N_CORES} cores"
    B_L = B // N_CORES
    shared = {k: inputs[k] for k in
              ("Wq", "bq", "Wk", "bk", "Wv", "bv", "Wp", "bp")}
    in_maps = []
    for i in range(N_CORES):
        m = dict(shared)
        m["x_base"] = np.ascontiguousarray(inputs["x_base"][i * B_L:(i + 1) * B_L])
        m["x_target"] = np.ascontiguousarray(inputs["x_target"][i * B_L:(i + 1) * B_L])
        in_maps.append(m)
    return in_maps


def kernel(**inputs):
    B = np.asarray(inputs["x_base"]).shape[0]
    B_L = B // N_CORES
    nc = _get_nc(B_L)
    in_maps = _make_in_maps(inputs)
    res = run_bass_kernel_spmd(nc, in_maps, core_ids=list(range(N_CORES)))
    return np.concatenate([r["out"] for r in res.results], axis=1)


# revision 36
# speedup vs baseline: 1.0334x; 1.0297x over previous
"""Trainium2 Bass kernel for nn_Attention_86217173500445.

Cross-attention block: shared QKV projections over two inputs (base/target),
4 attention streams (bb, tt, bt, tb), shared output projection.

Strategy: data-parallel over batch (B=32 -> 4 per core on 8 cores), weights
replicated, zero collectives.  Per-core compute is a fused bf16 pipeline
(fp32 PSUM accumulation everywhere, tolerance is 2e-2):
  - x transposed on-chip (PE transpose, fp32) to XT [C, S] bf16.
  - Q/K projections emit transposed QT/KT [C, 2src, S] bf16; V natural
    layout with a ones column (col 64) for even heads only, so their AV
    matmuls produce softmax row-sums for free; odd heads' row-sums come
    from ones_sel (all-ones lhsT column) matmuls accumulating straight
    into the rsum PSUM tile.
  - ScoresT [k, q] per (head, j), j-major; ACT exp -> bf16 feeds AV.
  - Odd heads' AV writes PSUM partitions 64:128 directly, so psum->sbuf
    OT copies are same-partition CASTs (no stream shuffle).
  - Even row-sum rows parked in an SBUF tile via 1-row ACT copies and
    gathered by K=1 matmuls at sigma end; reciprocal_approx_fast (DVE)
    overlaps the next sigma's scores/AV.
  - The PE instruction stream is software-pipelined across phases: only
    QK chunks m0/m1 + V(src0) run as an upfront block per batch; QK
    m2-5 + V(src1), the previous sigma's normalize+out-proj, and the
    next batch's x transposes are all interleaved into the scores/AV hh
    loops.  This keeps the PE dense through the ACT-exp latency and the
    softmax-denominator barrier, holding the HAM clock gate at K=8/8
    (the unpipelined fp32r version lost ~55% of its runtime to K=4/8
    throttling; this version loses one 3.4us HAM window per batch).
  - PSUM: 6-slot bank ring (tag "sm") + 2 rsum banks.
"""

import numpy as np

import concourse.bass as bass
import concourse.bacc as bacc
import concourse.mybir as mybir
import concourse.tile as tile
from concourse.bass_utils import run_bass_kernel_spmd
from concourse.masks import make_identity

FP32 = mybir.dt.float32
FP32R = mybir.dt.float32r
BF16 = mybir.dt.bfloat16
AF = mybir.ActivationFunctionType

H, DH, S, C = 12, 64, 197, 768
NCO = C // 128  # 6 channel chunks
SCALE = DH ** -0.5
S_TILES = [(0, 128), (128, 69)]
N_CHUNKS = [(0, 384), (384, 384)]  # out-proj/V-proj column chunks (6 heads)
# (key/value source, query source) -> output stream index; 0=base, 1=target
STREAM_IDX = {(0, 0): 0, (0, 1): 3, (1, 1): 1, (1, 0): 2}
N_CORES = 8
S2 = 2 * S  # query axis covers both query sources side by side


def _ap(base, free_dims):
    """AP with base's partition dim and explicit free [stride, size] dims."""
    return bass.AP(tensor=base.tensor, offset=base.offset,
                   ap=[list(base.ap[0])] + [list(d) for d in free_dims])


def build_nc(B_L):
    nc = bacc.Bacc("TRN2", target_bir_lowering=False, debug=False,
                   num_devices=N_CORES)

    x_in = {
        0: nc.dram_tensor("x_base", [B_L, S, C], FP32, kind="ExternalInput"),
        1: nc.dram_tensor("x_target", [B_L, S, C], FP32, kind="ExternalInput"),
    }
    w_dram, b_dram = {}, {}
    for nm in ("q", "k", "v", "p"):
        w_dram[nm] = nc.dram_tensor(f"W{nm}", [C, C], FP32, kind="ExternalInput")
        b_dram[nm] = nc.dram_tensor(f"b{nm}", [C], FP32, kind="ExternalInput")
    out_d = nc.dram_tensor("out", [4, B_L, S, C], FP32, kind="ExternalOutput")

    with tile.TileContext(nc) as tc:
        with (
            tc.tile_pool(name="const", bufs=1) as constp,
            tc.tile_pool(name="stage", bufs=6) as stagep,
            tc.tile_pool(name="wsb", bufs=1) as wp,
            tc.tile_pool(name="xt", bufs=2) as xtp,
            tc.tile_pool(name="qkv", bufs=2) as qkvp,
            tc.tile_pool(name="expp", bufs=4) as expp,
            tc.tile_pool(name="ot", bufs=2) as otp,
            tc.tile_pool(name="rpool", bufs=2) as rp,
            tc.tile_pool(name="y2", bufs=3) as y2p,
            tc.tile_pool(name="ps", bufs=6, space="PSUM") as ps,
            tc.tile_pool(name="psr", bufs=2, space="PSUM") as psr,
        ):
            # ---- constants ----
            ident = constp.tile([128, 128], FP32)
            make_identity(nc, ident)

            # E[h, c] = 1 iff channel c belongs to head h (normalize bcast)
            E_f32 = constp.tile([H, C], FP32)
            nc.gpsimd.memset(E_f32, 1.0)
            nc.gpsimd.affine_select(
                out=E_f32, in_=E_f32, compare_op=mybir.AluOpType.is_ge, fill=0.0,
                base=0, pattern=[[1, C]], channel_multiplier=-DH)
            nc.gpsimd.affine_select(
                out=E_f32, in_=E_f32, compare_op=mybir.AluOpType.is_ge, fill=0.0,
                base=DH - 1, pattern=[[-1, C]], channel_multiplier=DH)
            E_sb = constp.tile([H, C], FP32R)
            nc.vector.tensor_copy(out=E_sb, in_=E_f32)

            # E3[p, h, j] = (j == h): one-hot rows used (at partitions 63/64)
            # to gather each head's AV rowsum row into one [H, S2] psum
            E3_f32 = constp.tile([128, H, H], FP32)
            nc.gpsimd.memset(E3_f32, 0.0)
            for h in range(H):
                nc.gpsimd.memset(E3_f32[:, h, h:h + 1], 1.0)
            E3_sb = constp.tile([128, H, H], BF16)
            nc.vector.tensor_copy(out=E3_sb, in_=E3_f32)

            # ones_sel[p, i, j] = (j == 2i+1): all-ones column per odd head,
            # used as matmul lhsT to reduce exp over keys -> rowsum row 2i+1
            # of the rsum psum tile (odd heads carry no ones column in V).
            osel_f32 = constp.tile([128, H // 2, H], FP32)
            nc.gpsimd.memset(osel_f32, 0.0)
            for i in range(H // 2):
                nc.gpsimd.memset(osel_f32[:, i, 2 * i + 1:2 * i + 2], 1.0)
            ones_sel = constp.tile([128, H // 2, H], BF16)
            nc.vector.tensor_copy(out=ones_sel, in_=osel_f32)

            # fp32 ones used to write the bf16 ones-columns of V via DVE copy
            ones_c = constp.tile([128, H], FP32)
            nc.gpsimd.memset(ones_c, 1.0)

            # per-partition channel biases for the transposed Q/K outputs
            bqk_sb = {}
            for nm in ("q", "k"):
                t = constp.tile([128, NCO], FP32, name=f"b{nm}_sb")
                nc.gpsimd.dma_start(
                    out=t, in_=b_dram[nm].rearrange("(ko p) -> p ko", p=128))
                bqk_sb[nm] = t
            # biases broadcast along partitions for natural-layout outputs
            bbc_sb = {}
            for nm in ("v", "p"):
                t = constp.tile([128, C], FP32, name=f"b{nm}_bc")
                src_ap = b_dram[nm][:]
                bcast = bass.AP(tensor=src_ap.tensor, offset=src_ap.offset,
                                ap=[[0, 128]] + list(src_ap.ap))
                nc.gpsimd.dma_start(out=t, in_=bcast)
                bbc_sb[nm] = t

            # ---- prefetch batch-0 x tiles ahead of the weight loads so the
            # PE transposes can start immediately ----
            x_stage = {}
            for src in (0, 1):
                for (s0, s_sz) in S_TILES:
                    xs = stagep.tile([128, C], FP32, tag="stage", name="xs")
                    nc.sync.dma_start(out=xs[:s_sz, :],
                                      in_=x_in[src][0, s0:s0 + s_sz, :])
                    x_stage[(0, src, s0)] = xs

            # ---- weights: DMA fp32 then DVE-cast to bf16.  Batch-0's
            # transposes are emitted right after Wq so their DVE psum->sbuf
            # copies queue ahead of the remaining 18 weight casts (the
            # in-order DVE queue otherwise stalls the PE transposes behind
            # ~26us of weight DMA). ----
            W_sb = {}

            def emit_transpose(XT, tb, src, sti):
                s0, s_sz = S_TILES[sti]
                xs = x_stage.pop((tb, src, s0))
                for co in range(NCO):
                    pt = ps.tile([128, 128], FP32, tag="sm", name="pt")
                    nc.tensor.transpose(
                        pt[:, :s_sz], xs[:s_sz, co * 128:(co + 1) * 128],
                        ident[:s_sz, :s_sz])
                    nc.vector.tensor_copy(
                        out=XT[:, co, src, s0:s0 + s_sz], in_=pt[:, :s_sz])


            def emit_b0_transposes(XT):
                for src in (0, 1):
                    for sti in range(len(S_TILES)):
                        emit_transpose(XT, 0, src, sti)

            for nm in ("q", "k", "v", "p"):
                W_sb[nm] = wp.tile([128, NCO, C], BF16, tag=f"w{nm}",
                                   name=f"W{nm}_sb")
                for ko in range(NCO):
                    st = stagep.tile([128, C], FP32, tag="wstage", name="wst",
                                     bufs=3)
                    nc.sync.dma_start(out=st,
                                      in_=w_dram[nm][ko * 128:(ko + 1) * 128, :])
                    nc.vector.tensor_copy(out=W_sb[nm][:, ko, :], in_=st)
                if nm == "q":
                    XT0_early = xtp.tile([128, NCO, 2, S], BF16, tag="xt",
                                         name="XT0_early")
                    emit_b0_transposes(XT0_early)

            def make_tail(tb, tsigma, OT_raw_p, rr32_p):
                """Normalize + out-proj for a finished sigma, sliced into
                H//2 piece-lists for interleaving into the next sigma's
                scores/AV loop."""
                OT = otp.tile([128, NCO, S2], BF16, tag="ot")
                ycur = [None]

                def pr_piece(co):
                    pr = ps.tile([128, S2], FP32, tag="sm", name="pr")
                    nc.tensor.matmul(
                        pr[:], lhsT=E_sb[:, co * 128:(co + 1) * 128],
                        rhs=rr32_p[:], start=True, stop=True)
                    nc.vector.tensor_mul(out=OT[:, co, :],
                                         in0=OT_raw_p[:, co, :], in1=pr[:])

                def py_piece(g):
                    qs, sti_, nci = g // 4, (g // 2) % 2, g % 2
                    s0, s_sz = S_TILES[sti_]
                    n0, n_sz = N_CHUNKS[nci]
                    if nci == 0:
                        ycur[0] = y2p.tile([128, C], FP32, tag="y2",
                                           name="y")
                    y = ycur[0]
                    py = ps.tile([128, 384], FP32, tag="sm", name="py")
                    for k in range(NCO):
                        nc.tensor.matmul(
                            py[:s_sz, :n_sz],
                            lhsT=OT[:, k, qs * S + s0:qs * S + s0 + s_sz],
                            rhs=W_sb["p"][:, k, n0:n0 + n_sz],
                            start=(k == 0), stop=(k == NCO - 1))
                    nc.vector.tensor_add(
                        out=y[:s_sz, n0:n0 + n_sz], in0=py[:s_sz, :n_sz],
                        in1=bbc_sb["p"][:s_sz, n0:n0 + n_sz])
                    if nci == 1:
                        stream = STREAM_IDX[(tsigma, qs)]
                        nc.sync.dma_start(
                            out=out_d[stream, tb, s0:s0 + s_sz, :],
                            in_=y[:s_sz, :])

                def P(fn, *a):
                    return lambda: fn(*a)

                return [[P(pr_piece, 0), P(pr_piece, 1), P(pr_piece, 2)],
                        [P(pr_piece, 3), P(pr_piece, 4), P(pr_piece, 5),
                         P(py_piece, 0)],
                        [P(py_piece, 1), P(py_piece, 2)],
                        [P(py_piece, 3), P(py_piece, 4)],
                        [P(py_piece, 5)],
                        [P(py_piece, 6)],
                        [P(py_piece, 7)]]

            def make_transpose_pieces(tb):
                """Next batch's x transposes, sliced for interleaving into
                the current sigma's hh loop (transposes are HAM-invisible;
                standalone they demote the PE clock every batch)."""
                XT = xtp.tile([128, NCO, 2, S], BF16, tag="xt")

                def tp(src, sti):
                    emit_transpose(XT, tb, src, sti)

                def P(fn, *a):
                    return lambda: fn(*a)

                return XT, [[], [P(tp, 0, 0)], [P(tp, 0, 1)],
                            [P(tp, 1, 0)], [P(tp, 1, 1)], [], []]

            pending = None
            XT = XT0_early

            for b in range(B_L):
                # prefetch next batch's x tiles (overlaps this batch)
                if b + 1 < B_L:
                    for src in (0, 1):
                        for (s0, s_sz) in S_TILES:
                            xs = stagep.tile([128, C], FP32, tag="stage",
                                             name="xs")
                            nc.sync.dma_start(
                                out=xs[:s_sz, :],
                                in_=x_in[src][b + 1, s0:s0 + s_sz, :])
                            x_stage[(b + 1, src, s0)] = xs

                # ---- Q/K/V projections.  Only QK chunks m0/m1 and V(src0)
                # are emitted as an upfront block; QK m2-5 and V(src1) are
                # interleaved into sigma-0's scores/AV loop below. ----
                QT = qkvp.tile([128, NCO, 2, S], BF16, tag="qt")
                KT = qkvp.tile([128, NCO, 2, S], BF16, tag="kt")
                V_sb = qkvp.tile([128, 2, 2, H, DH + 1], BF16, tag="v")
                XT_b = XT

                def emit_qk(nm, m, XTl=None, QTl=None, KTl=None):
                    XTl = XT_b if XTl is None else XTl
                    OUT = (QT if QTl is None else QTl) if nm == "q" else \
                          (KT if KTl is None else KTl)
                    pp = [ps.tile([128, S], FP32, tag="sm", name="pp0"),
                          ps.tile([128, S], FP32, tag="sm", name="pp1")]
                    for k in range(NCO):
                        for src in (0, 1):
                            nc.tensor.matmul(
                                pp[src][:],
                                lhsT=W_sb[nm][:, k, m * 128:(m + 1) * 128],
                                rhs=XTl[:, k, src, :],
                                start=(k == 0), stop=(k == NCO - 1))
                    for src in (0, 1):
                        nc.scalar.activation(
                            out=OUT[:, m, src, :], in_=pp[src][:],
                            func=AF.Identity,
                            bias=bqk_sb[nm][:, m:m + 1], scale=1.0)

                def emit_v(src, sti, nci):
                    s0, s_sz = S_TILES[sti]
                    n0, n_sz = N_CHUNKS[nci]
                    pv = ps.tile([128, 384], FP32, tag="sm", name="pv")
                    for k in range(NCO):
                        nc.tensor.matmul(
                            pv[:s_sz, :n_sz],
                            lhsT=XT_b[:, k, src, s0:s0 + s_sz],
                            rhs=W_sb["v"][:, k, n0:n0 + n_sz],
                            start=(k == 0), stop=(k == NCO - 1))
                    h0, nh = n0 // DH, n_sz // DH
                    nc.vector.tensor_add(
                        out=V_sb[:s_sz, src, sti, h0:h0 + nh, 0:DH],
                        in0=pv[:s_sz, :n_sz].rearrange(
                            "p (h d) -> p h d", d=DH),
                        in1=bbc_sb["v"][:s_sz, n0:n0 + n_sz].rearrange(
                            "p (h d) -> p h d", d=DH))
                    if nci == len(N_CHUNKS) - 1:
                        dst_e = _ap(V_sb[0:s_sz, src, sti, 0, DH],
                                    [[2 * (DH + 1), H // 2]])
                        nc.vector.tensor_copy(out=dst_e,
                                              in_=ones_c[:s_sz, :H // 2])

                for m in (0, 1):
                    for nm in ("q", "k"):
                        emit_qk(nm, m)
                for sti in range(len(S_TILES)):
                    for nci in range(len(N_CHUNKS)):
                        emit_v(0, sti, nci)

                # ---- attention.  scores+AV+rowsum per sigma, with the
                # PREVIOUS sigma's normalize+out-proj matmuls interleaved
                # into the hh loop: the out-proj work fills the PE bubbles
                # left by ACT exp latency (keeps the HAM clock gate at 8/8),
                # and the softmax-denominator reciprocal gets a whole
                # scores/AV phase to complete off the critical path. ----
                for sigma in (0, 1):
                    OT_raw = otp.tile([128, NCO, S2], BF16, tag="otraw")
                    rsball = rp.tile([DH + 1, H, S2], BF16, tag="rsb")
                    rsum = psr.tile([128, S2], FP32, tag="rsum")
                    ets = {}

                    def emit_scores(hh):
                        for sti in range(len(S_TILES)):
                            ets[(hh, sti)] = expp.tile([128, 2, S2], BF16,
                                                       tag="exp", name="et")
                        # j-major so AV j=0's two exp deps complete first
                        for j in (0, 1):
                            for sti, (s0, s_sz) in enumerate(S_TILES):
                                et = ets[(hh, sti)]
                                psc = ps.tile([128, S2], FP32, tag="sm",
                                              name="psc")
                                nc.tensor.matmul(
                                    psc[:s_sz, :],
                                    lhsT=KT[j * DH:(j + 1) * DH, hh, sigma,
                                            s0:s0 + s_sz],
                                    rhs=QT[j * DH:(j + 1) * DH, hh, :, :],
                                    start=True, stop=True)
                                nc.scalar.activation(
                                    out=et[:s_sz, j, :], in_=psc[:s_sz, :],
                                    func=AF.Exp, scale=float(SCALE))

                    def emit_av(hh):
                        et = [ets.pop((hh, 0)), ets.pop((hh, 1))]
                        for j in (0, 1):
                            h = 2 * hh + j
                            pav = ps.tile([128, S2], FP32, tag="sm",
                                          name="pav")
                            if j == 0:
                                # even: ones col in V -> rowsum at row DH
                                for sti, (s0, s_sz) in enumerate(S_TILES):
                                    nc.tensor.matmul(
                                        pav[0:DH + 1, :],
                                        lhsT=V_sb[:s_sz, sigma, sti, h, :],
                                        rhs=et[sti][:s_sz, j, :],
                                        start=(sti == 0), stop=(sti == 1))
                                nc.scalar.copy(out=rsball[DH:DH + 1, h, :],
                                               in_=pav[DH:DH + 1, :])
                                nc.vector.tensor_copy(
                                    out=OT_raw[0:DH, hh, :],
                                    in_=pav[0:DH, :])
                            else:
                                # odd: data straight to partitions 64:128;
                                # rowsum via ones_sel into the rsum accum
                                for sti, (s0, s_sz) in enumerate(S_TILES):
                                    nc.tensor.matmul(
                                        pav[DH:2 * DH, :],
                                        lhsT=V_sb[:s_sz, sigma, sti, h, 0:DH],
                                        rhs=et[sti][:s_sz, j, :],
                                        start=(sti == 0), stop=(sti == 1))
                                for sti, (s0, s_sz) in enumerate(S_TILES):
                                    nc.tensor.matmul(
                                        rsum[:H, :],
                                        lhsT=ones_sel[:s_sz, hh, :],
                                        rhs=et[sti][:s_sz, j, :],
                                        start=(hh == 0 and sti == 0),
                                        stop=False)
                                nc.vector.tensor_copy(
                                    out=OT_raw[DH:2 * DH, hh, :],
                                    in_=pav[DH:2 * DH, :])

                    pieces = (make_tail(*pending) if pending is not None
                              else [[] for _ in range(H // 2 + 1)])
                    if sigma == 0:
                        def QP(nm, m):
                            return lambda: emit_qk(nm, m)

                        def VP(sti, nci):
                            return lambda: emit_v(1, sti, nci)

                        for i, m in enumerate((2, 3, 4, 5)):
                            pieces[i] = [QP("q", m), QP("k", m)] + pieces[i]
                        pieces[2].append(VP(0, 0))
                        pieces[3].append(VP(1, 0))
                        pieces[4].append(VP(0, 1))
                        pieces[6].append(VP(1, 1))
                    elif b + 1 < B_L:
                        XT_next, tpieces = make_transpose_pieces(b + 1)
                        pieces = [a + t for a, t in zip(pieces, tpieces)]
                    for hh in range(H // 2):
                        emit_scores(hh)
                        for fn in pieces[hh]:
                            fn()
                        if hh:
                            emit_av(hh - 1)
                    for fn in pieces[H // 2]:
                        fn()
                    emit_av(H // 2 - 1)

                    # gather the 6 even rowsum rows -> rsum[0:H] (K=1)
                    for h in range(0, H, 2):
                        nc.tensor.matmul(
                            rsum[:H, :], lhsT=E3_sb[DH:DH + 1, h, :],
                            rhs=rsball[DH:DH + 1, h, :],
                            start=False, stop=(h == H - 2))
                    rr32 = rp.tile([H, S2], FP32, tag="rrf")
                    nc.vector.reciprocal_approx_fast(out=rr32,
                                                     in_=rsum[:H, :])
                    rr = rp.tile([H, S2], FP32R, tag="rr")
                    with nc.allow_low_precision(reason="softmax denom"):
                        nc.vector.tensor_copy(out=rr, in_=rr32)
                    pending = (b, sigma, OT_raw, rr)

                if b + 1 < B_L:
                    XT = XT_next

            # last sigma's normalize+out-proj (nothing left to interleave)
            for piece in make_tail(*pending):
                for fn in piece:
                    fn()
    nc.compile()
    return nc


_NC_CACHE = {}


def _get_nc(B_L):
    if B_L not in _NC_CACHE:
        _NC_CACHE[B_L] = build_nc(B_L)
    return _NC_CACHE[B_L]


def _make_in_maps(inputs):
    inputs = {k: np.ascontiguousarray(np.asarray(v), dtype=np.float32)
              for k, v in inputs.items()}
    B = inputs["x_base"].shape[0]
    assert B % N_CORES == 0, f"batch {B} not divisible by {N_CORES} cores"
    B_L = B // N_CORES
    shared = {k: inputs[k] for k in
              ("Wq", "bq", "Wk", "bk", "Wv", "bv", "Wp", "bp")}
    in_maps = []
    for i in range(N_CORES):
        m = dict(shared)
        m["x_base"] = np.ascontiguousarray(inputs["x_base"][i * B_L:(i + 1) * B_L])
        m["x_target"] = np.ascontiguousarray(inputs["x_target"][i * B_L:(i + 1) * B_L])
        in_maps.append(m)
    return in_maps


def kernel(**inputs):
    B = np.asarray(inputs["x_base"]).shape[0]
    B_L = B // N_CORES
    nc = _get_nc(B_L)
    in_maps = _make_in_maps(inputs)
    res = run_bass_kernel_spmd(nc, in_maps, core_ids=list(range(N_CORES)))
    return np.concatenate([r["out"] for r in res.results], axis=1)
